# revision 1
# baseline (speedup 1.0000x reference)
"""CTC batch loss (Keras convention, blank = C-1) on 8 Trainium2 NeuronCores.

Strategy (pure data parallel, 128 examples per core = 128 SBUF partitions):
  * Prob-domain scaled forward DP (mathematically identical to the reference's
    log-space DP, including the exact log(p + 1e-7) epsilon, which is folded
    into the gather as E = onehot + eps so gathered values are p + eps).
  * Label gather via per-example one-hot matmul on the TensorEngine:
    bf16 pipeline: gpsimd cast-DMA (f32->bf16 inline), XBAR dma transpose
    (t,c)->(c,t), matmul E'^T . X^T -> PSUM f32 [65, t], DVE escape copy,
    DMA re-layout into batch-partitioned p_store (f32).
  * Serial DP over T: 4 DVE ops/step with guard columns making the s-1/s-2
    shifts plain AP offsets; the skip-transition mask runs on GPSIMD off the
    critical path; rescale every 8 steps, log-corrections collected in a
    strip and reduced once at the end.
"""

import sys
from contextlib import ExitStack

import numpy as np

for _p in ("/opt/trn_rl_repo",):
    if _p not in sys.path:
        sys.path.insert(0, _p)

import concourse.bass as bass
import concourse.tile as tile
from concourse import mybir
from concourse.bass_utils import run_bass_kernel_spmd

# Problem constants (hardcoded per spec nn_CTC_55808805045003)
B, T, C, L = 1024, 256, 128, 64
NCORES = 8
BL = B // NCORES          # 128 examples per core
S = 2 * L + 1             # 129 extended labels
NS = L + 1                # 65 gather columns (64 labels + blank)
EPS = 1e-7
CH = 128                  # time chunk
NCH = T // CH             # 2
RESC = 8                  # rescale period
GRP = 16                  # examples per cast-DMA group

f32 = mybir.dt.float32
bf16 = mybir.dt.bfloat16
f16 = mybir.dt.float16
i32 = mybir.dt.int32

# gather-pipeline storage dtype for probabilities (bf16 or f16).
# f16 has a 10-bit mantissa (4x finer than bf16); p < 6e-5 lands in f16
# subnormals, which numpy handles exactly and HW needs to not flush.
import os as _os

GDT_NAME = _os.environ.get("CTC_GDT", "f16")
GDT = {"bf16": bf16, "f16": f16}[GDT_NAME]
GDT_NP = {"bf16": None, "f16": np.float16}[GDT_NAME]
ADD = mybir.AluOpType.add
MULT = mybir.AluOpType.mult
ISEQ = mybir.AluOpType.is_equal
NEQ = mybir.AluOpType.not_equal
AX_X = mybir.AxisListType.X
AFT = mybir.ActivationFunctionType


def _body(tc, loss_ap, yp, lab_ap, e_ap):
    nc = tc.nc
    with ExitStack() as ctx:
        const = ctx.enter_context(tc.tile_pool(name="const", bufs=1))
        dstage = ctx.enter_context(tc.tile_pool(name="dstage", bufs=1, space="DRAM"))
        xtp = ctx.enter_context(tc.tile_pool(name="xt", bufs=8))
        gps = ctx.enter_context(tc.tile_pool(name="gpsum", bufs=2, space="PSUM"))
        gsb = ctx.enter_context(tc.tile_pool(name="gsb", bufs=2))
        tiny = ctx.enter_context(tc.tile_pool(name="tiny", bufs=6))

        # ---- label-derived constants (host-computed, DMA'd in) ----
        Eall = const.tile([128, BL * NS], GDT)
        nc.sync.dma_start(Eall[:], e_ap[:, :])
        m_odd = const.tile([128, L], f32)
        nc.sync.dma_start(m_odd[:], lab_ap[:, :])

        # per-chunk gathered probs: p_store[ch][b, s*CH + t] = p(b, ch*CH+t, ext65[b,s]) + eps
        p_stores = [
            const.tile([128, NS * CH], f32, name=f"p_store{ch}") for ch in range(NCH)
        ]
        ps3 = [
            p_stores[ch][:].rearrange("p (s t) -> p s t", s=NS) for ch in range(NCH)
        ]

        ystages = [
            dstage.tile([BL, CH, C], GDT, name=f"ystage{ch}") for ch in range(NCH)
        ]

        GT = 8  # examples per XBAR transpose / per PSUM+gs tile / per p_store DMA

        def gather_chunk(ch):
            t0 = ch * CH
            ystage = ystages[ch]
            for g in range(BL // GRP):
                # f32 -> f16 cast inline in the DMA (SWDGE), DRAM -> DRAM
                nc.gpsimd.dma_start(
                    ystage[g * GRP : (g + 1) * GRP, :, :],
                    yp[g * GRP : (g + 1) * GRP, t0 : t0 + CH, :],
                )
            for g in range(BL // GT):
                b0 = g * GT
                # one XBAR transpose covers GT examples: [GT*CH, C] -> [C, GT*CH]
                xt = xtp.tile([C, GT * CH], GDT)
                nc.sync.dma_start_transpose(
                    xt[:],
                    ystage[b0 : b0 + GT, :, :].rearrange("b t c -> (b t) c"),
                )
                gp = gps.tile([NS, GT * CH], f32)
                for i in range(GT):
                    b = b0 + i
                    nc.tensor.matmul(
                        gp[:, i * CH : (i + 1) * CH],
                        Eall[:, b * NS : (b + 1) * NS],
                        xt[:, i * CH : (i + 1) * CH],
                        start=True,
                        stop=True,
                    )
                gs = gsb.tile([NS, GT * CH], f32)
                nc.vector.tensor_copy(gs[:], gp[:])
                # per-example re-layout into batch partitions, spread across
                # both HWDGE queues (SP + ACT)
                for i in range(GT):
                    b = b0 + i
                    eng = nc.scalar if (b % 2) else nc.sync
                    eng.dma_start(
                        ps3[ch][b : b + 1, :, :], gs[:, i * CH : (i + 1) * CH]
                    )

        for ch in range(NCH):
            gather_chunk(ch)

        # ---- DP state ----
        # alpha cols: 0,1 = zero guards; 2..130 = s=0..128; 131 pad
        alpha = const.tile([128, 132], f32)
        u = const.tile([128, 132], f32)
        v_odd = const.tile([128, 64], f32)
        aM = const.tile([128, 66], f32)  # col 0 guard; 1..64 = masked odd alphas
        strip = const.tile([128, 32], f32)

        nc.vector.memset(alpha[:], 0.0)
        nc.vector.memset(aM[:], 0.0)

        # t = 0 init: alpha[s=0] = p_blank(t=0), alpha[s=1] = p_lab0(t=0)
        nc.vector.tensor_copy(alpha[:, 2:3], ps3[0][:, NS - 1 : NS, 0:1].squeeze(2))
        nc.vector.tensor_copy(alpha[:, 3:4], ps3[0][:, 0:1, 0:1].squeeze(2))
        # aM[1+j'] = alpha_odd[j'] * m_dest[j'], m_dest[j'] = (lab[j'+1] != lab[j'])
        nc.gpsimd.tensor_tensor(aM[:, 1:2], alpha[:, 3:4], m_odd[:, 0:1], MULT)

        # running rescale factor, applied inside opC's scalar slot; 1.0 except
        # on the step right after each row-sum snapshot
        r_ap = const.tile([128, 1], f32)
        nc.vector.memset(r_ap[:], 1.0)

        k_resc = 0
        for t in range(1, T):
            p3 = ps3[t // CH]
            tt = t % CH
            p_lab = p3[:, 0:64, tt : tt + 1].squeeze(2)
            p_bl = p3[:, 64:65, tt : tt + 1]
            snap = t % RESC == 0  # snapshot row-sum this step, rescale next step
            # u[s] = alpha[s] + alpha[s-1]
            nc.vector.tensor_tensor(u[:, 2:131], alpha[:, 2:131], alpha[:, 1:130], ADD)
            # v_odd[j] = u[2j+3] + aM_prev[j-1]
            nc.vector.tensor_tensor(v_odd[:], u[:, 3:130:2], aM[:, 0:64], ADD)
            # alpha_odd = (v_odd * r) * p_lab
            nc.vector.scalar_tensor_tensor(
                alpha[:, 3:130:2], v_odd[:], r_ap[:], p_lab, MULT, MULT
            )
            # alpha_even = (u_even * r) * p_blank
            nc.vector.tensor_scalar(
                alpha[:, 2:131:2], u[:, 2:131:2], r_ap[:], p_bl, MULT, MULT
            )
            # masked odd alphas for the next step's skip term (off critical path);
            # source j'=0..62 feeds destination j'+1, gated by m_dest[j']
            nc.gpsimd.tensor_tensor(
                aM[:, 1:64], alpha[:, 3:128:2], m_odd[:, 0:63], MULT
            )
            if snap:
                cs = tiny.tile([128, 1], f32)
                nc.vector.tensor_reduce(cs[:], alpha[:, 2:131], AX_X, ADD)
                nc.vector.reciprocal(r_ap[:], cs[:])
                nc.scalar.activation(strip[:, k_resc : k_resc + 1], cs[:], AFT.Ln)
                k_resc += 1
            elif t % RESC == 1 and t > 1:
                # r was consumed by this step's opC ops; reset to 1.0
                nc.gpsimd.memset(r_ap[:], 1.0)

        # loss = -(sum_k log c_k + log(alpha[S-1] + alpha[S-2]))
        lik = tiny.tile([128, 1], f32)
        nc.vector.tensor_tensor(lik[:], alpha[:, 129:130], alpha[:, 130:131], ADD)
        nc.scalar.activation(strip[:, 31:32], lik[:], AFT.Ln)
        assert k_resc == 31
        slog = tiny.tile([128, 1], f32)
        nc.vector.tensor_reduce(slog[:], strip[:], AX_X, ADD)
        lout = tiny.tile([128, 1], f32)
        nc.vector.tensor_scalar(lout[:], slog[:], -1.0, None, MULT)
        nc.sync.dma_start(loss_ap[:, :], lout[:])


def build_nc():
    nc = bass.Bass("TRN2", target_bir_lowering=False, debug=False)
    yp = nc.dram_tensor("y_pred", [BL, T, C], f32, kind="ExternalInput").ap()
    lab = nc.dram_tensor("m_odd", [BL, L], f32, kind="ExternalInput").ap()
    e_in = nc.dram_tensor("e_all", [128, BL * NS], GDT, kind="ExternalInput").ap()
    loss = nc.dram_tensor("loss", [BL, 1], f32, kind="ExternalOutput").ap()
    with tile.TileContext(nc) as tc:
        _body(tc, loss, yp, lab, e_in)
    return nc


def host_label_consts(y_true):
    """E' one-hot (+eps, bf16) and skip-mask, per core: pure functions of labels."""
    import ml_dtypes

    lab = np.asarray(y_true).astype(np.int64)  # [B, L]
    outs = []
    for i in range(NCORES):
        lb = lab[i * BL : (i + 1) * BL]  # [128, 64]
        ext = np.concatenate(
            [lb, np.full((BL, 1), C - 1, np.int64)], axis=1
        )  # [128, 65]
        e = (np.arange(128)[:, None, None] == ext[None, :, :]).astype(np.float32)
        npdt = GDT_NP or ml_dtypes.bfloat16
        e = (e + EPS).astype(npdt).reshape(128, BL * NS)
        # destination-indexed skip mask: m[j'] = (lab[j'+1] != lab[j']), j'=0..62
        m = np.zeros((BL, L), np.float32)
        m[:, 0:63] = (lb[:, 1:] != lb[:, :-1]).astype(np.float32)
        outs.append((e, m))
    return outs


_CACHE = {}

# --- BIR legalizer -----------------------------------------------------------
# This container's walrus encodes at most ONE sync wait on SP-queue
# instruction classes (PSEUDO_DMA_DIRECT2D / XPOSE / CTRL): "Too many sync
# wait commands". Tile freely emits >=2 waits per instruction. Split the
# extras onto NoOps inserted just before (same engine stream => semantics
# preserved, waits satisfied in order).
_SPLIT_OPS = {"DMACopy", "DmaTransposeAnt", "DMAGatherAnt", "Drain", "NoOp"}


def _legalize_bir(bir_bytes):
    import orjson

    d = orjson.loads(bir_bytes)
    n_new = 0
    for fn in d.get("functions", []):
        for blk in fn.get("blocks", []):
            insts = blk.get("instructions")
            if not insts:
                continue
            out = []
            for ins in insts:
                si = ins.get("sync_info")
                if si:
                    waits = si.get("on_wait") or []
                    if len(waits) > 1:
                        for w in waits[:-1]:
                            n_new += 1
                            out.append(
                                {
                                    "debug": ins.get("debug", 0),
                                    "engine": ins["engine"],
                                    "ins": [],
                                    "outs": [],
                                    "name": f"ZW-{n_new}",
                                    "opcode": "NoOp",
                                    "sync_info": {"on_wait": [w], "on_update": []},
                                }
                            )
                        si["on_wait"] = [waits[-1]]
                out.append(ins)
            blk["instructions"] = out
    return orjson.dumps(d)


def _install_bir_legalizer():
    import concourse.bass2jax as b2j

    if getattr(b2j, "_ctc_legalizer_installed", False):
        return
    orig = b2j.compile_bir_kernel

    def wrapper(bir_json, tmpdir, neff_name="file.neff"):
        bir_json = _legalize_bir(bir_json)
        return orig(bir_json, tmpdir, neff_name=neff_name)

    b2j.compile_bir_kernel = wrapper
    b2j._ctc_legalizer_installed = True


def kernel(y_true, y_pred):
    assert y_pred.shape == (B, T, C) and y_true.shape == (B, L)
    _install_bir_legalizer()
    nc = _CACHE.get("nc")
    if nc is None:
        nc = _CACHE["nc"] = build_nc()
    yp = np.ascontiguousarray(y_pred, dtype=np.float32)
    consts = host_label_consts(y_true)
    in_maps = [
        {
            "y_pred": yp[i * BL : (i + 1) * BL],
            "m_odd": consts[i][1],
            "e_all": consts[i][0],
        }
        for i in range(NCORES)
    ]
    res = run_bass_kernel_spmd(nc, in_maps, list(range(NCORES)))
    out = np.concatenate([res.results[i]["loss"] for i in range(NCORES)], axis=0)
    return out.astype(np.float32)



# revision 14
# speedup vs baseline: 1.9579x; 1.9579x over previous
"""CTC batch loss (Keras convention, blank = C-1) on 8 Trainium2 NeuronCores.

Strategy (pure data parallel, 128 examples per core = 128 SBUF partitions):
  * Prob-domain scaled forward DP, identical math to the reference's
    log-space DP including the log(p + 1e-7) epsilon (folded into the
    gather as E = onehot + eps; softmax rows sum to 1 so the gathered
    value is exactly p + eps).
  * Gather via per-example one-hot matmul on the TensorEngine: y_pred
    arrives host-cast to f16 in t-chunk-major layout; XBAR dma-transpose
    (t,c)->(c,t); matmul E^T . X^T -> PSUM f32 [65, t]; ACT-engine escape
    copy casting to bf16; DMA re-layout into batch-partitioned p_store.
  * Serial DP over T in bf16 with a split even/odd contiguous layout so
    most hot ops run in the DVE 2x/4x packed perf modes: 6 DVE ops per
    step, no cross-engine hops; rescale every 8 steps, log-corrections
    collected in a strip and reduced once at the end.
"""

import sys
from contextlib import ExitStack

import numpy as np

for _p in ("/opt/trn_rl_repo",):
    if _p not in sys.path:
        sys.path.insert(0, _p)

import concourse.bass as bass
import concourse.tile as tile
from concourse import mybir
from concourse.bass_utils import run_bass_kernel_spmd

# Problem constants (hardcoded per spec nn_CTC_55808805045003)
B, T, C, L = 1024, 256, 128, 64
NCORES = 8
BL = B // NCORES          # 128 examples per core
S = 2 * L + 1             # 129 extended labels
NS = L + 1                # 65 gather columns (64 labels + blank)
EPS = 1e-7
CH = 128                  # time chunk
NCH = T // CH             # 2
RESC = 8                  # rescale period
GT = 8                    # examples per transpose/matmul-group/relayout DMA

f32 = mybir.dt.float32
bf16 = mybir.dt.bfloat16
f16 = mybir.dt.float16

ADD = mybir.AluOpType.add
MULT = mybir.AluOpType.mult
AX_X = mybir.AxisListType.X
AFT = mybir.ActivationFunctionType


def _body(tc, loss_ap, yp16, e_ap, m_ap):
    nc = tc.nc
    with ExitStack() as ctx:
        const = ctx.enter_context(tc.tile_pool(name="const", bufs=1))
        xtp = ctx.enter_context(tc.tile_pool(name="xt", bufs=4))
        gps = ctx.enter_context(tc.tile_pool(name="gpsum", bufs=3, space="PSUM"))
        gsb = ctx.enter_context(tc.tile_pool(name="gsb", bufs=3))
        tiny = ctx.enter_context(tc.tile_pool(name="tiny", bufs=6))

        # ---- label-derived constants (host-computed, DMA'd in) ----
        Eall = const.tile([128, BL * NS], f16)
        nc.sync.dma_start(Eall[:], e_ap[:, :])
        m_odd = const.tile([128, 63], bf16)
        nc.sync.dma_start(m_odd[:], m_ap[:, :])

        # per-chunk gathered probs (bf16): p_store[ch][b, s*CH + t]
        p_stores = [
            const.tile([128, NS * CH], bf16, name=f"p_store{ch}") for ch in range(NCH)
        ]
        ps3 = [
            p_stores[ch][:].rearrange("p (s t) -> p s t", s=NS) for ch in range(NCH)
        ]

        def gather_chunk(ch):
            for g in range(BL // GT):
                b0 = g * GT
                r0 = ch * BL + b0
                eng_t = nc.sync if (g % 2 == 0) else nc.scalar
                # one XBAR transpose covers GT examples: [GT*CH, C] -> [C, GT*CH]
                xt = xtp.tile([C, GT * CH], f16)
                eng_t.dma_start_transpose(
                    xt[:],
                    yp16[r0 : r0 + GT, :, :].rearrange("b t c -> (b t) c"),
                )
                gp = gps.tile([NS, GT * CH], f32)
                for i in range(GT):
                    b = b0 + i
                    # out[s, t] = sum_c E[c, s] * xt[c, t]
                    nc.tensor.matmul(
                        gp[:, i * CH : (i + 1) * CH],
                        Eall[:, b * NS : (b + 1) * NS],
                        xt[:, i * CH : (i + 1) * CH],
                        start=True,
                        stop=True,
                    )
                # escape PSUM f32 -> SBUF bf16 on the ACT engine (idle during DP)
                gs = gsb.tile([NS, GT * CH], bf16)
                nc.scalar.copy(gs[:], gp[:])
                # per-example re-layout into batch partitions, split across
                # the SP and ACT HWDGE queues
                for i in range(GT):
                    b = b0 + i
                    eng = nc.scalar if (b % 2) else nc.sync
                    eng.dma_start(
                        ps3[ch][b : b + 1, :, :], gs[:, i * CH : (i + 1) * CH]
                    )

        for ch in range(NCH):
            gather_chunk(ch)

        # ---- DP state (bf16, split even/odd contiguous layout) ----
        # alpha cols: 0 = zero guard; 1..65 = even states i=0..64 (s=2i);
        # 66 = zero guard; 67..130 = odd states j=0..63 (s=2j+1); 131 pad
        alpha = const.tile([128, 132], bf16)
        AM = const.tile([128, 64], bf16)   # AM[k] = mask*alpha_odd[k-1]; AM[0]=0
        uo = const.tile([128, 64], bf16)
        vo = const.tile([128, 64], bf16)
        strip = const.tile([128, 32], f32)
        r_ap = tiny.tile([128, 1], f32)
        # f32 copy of the blank row per chunk: tensor_scalar's scalar operand
        # must be f32
        pbl = [const.tile([128, CH], f32, name=f"pbl{ch}") for ch in range(NCH)]
        for ch in range(NCH):
            nc.vector.tensor_copy(pbl[ch][:], ps3[ch][:, NS - 1 : NS, :].squeeze(1))

        nc.vector.memset(alpha[:], 0.0)
        nc.vector.memset(AM[:], 0.0)

        # t = 0 init: alpha[s=0] = p_blank(0), alpha[s=1] = p_lab0(0),
        # AM[1] = m0 * alpha_odd0
        nc.vector.tensor_copy(alpha[:, 1:2], ps3[0][:, NS - 1 : NS, 0:1].squeeze(2))
        nc.vector.tensor_copy(alpha[:, 67:68], ps3[0][:, 0:1, 0:1].squeeze(2))
        nc.vector.tensor_tensor(AM[:, 1:2], alpha[:, 67:68], m_odd[:, 0:1], MULT)

        V = nc.vector
        k_resc = 0
        for t in range(1, T):
            ch, tt = divmod(t, CH)
            ps = ps3[ch]
            pl = ps[:, 0:64, tt : tt + 1].squeeze(2)  # [128, 64] labels
            pb = pbl[ch][:, tt : tt + 1]              # [128, 1] blank (f32)
            # uo[j] = alpha_odd[j] + alpha_even[j]
            V.tensor_tensor(uo[:], alpha[:, 67:131], alpha[:, 1:65], ADD)
            # vo[j] = uo[j] + AM[j]
            V.tensor_tensor(vo[:], uo[:], AM[:], ADD)
            # even' = even + odd shifted (before odd is overwritten)
            V.tensor_tensor(alpha[:, 1:66], alpha[:, 1:66], alpha[:, 66:131], ADD)
            # odd' = vo * p_lab
            V.tensor_tensor(alpha[:, 67:131], vo[:], pl, MULT)
            # even' *= p_blank (per-partition scalar -> 4x mode)
            V.tensor_scalar(alpha[:, 1:66], alpha[:, 1:66], pb, None, MULT)
            # AM' for next step: mask * alpha_odd' shifted
            V.tensor_tensor(AM[:, 1:64], alpha[:, 67:130], m_odd[:], MULT)
            if t % RESC == 0 and t < T - RESC + 1:
                V.tensor_reduce(strip[:, k_resc : k_resc + 1], alpha[:, 1:131], AX_X, ADD)
                V.reciprocal(r_ap[:], strip[:, k_resc : k_resc + 1])
                V.tensor_scalar(alpha[:, 1:131], alpha[:, 1:131], r_ap[:], None, MULT)
                V.tensor_scalar(AM[:], AM[:], r_ap[:], None, MULT)
                k_resc += 1

        # loss = -(sum_k log c_k + log(alpha[S-1] + alpha[S-2]))
        assert k_resc == 31
        V.tensor_tensor(strip[:, 31:32], alpha[:, 65:66], alpha[:, 130:131], ADD)
        lns = tiny.tile([128, 32], f32)
        nc.scalar.activation(lns[:], strip[:], AFT.Ln)
        slog = tiny.tile([128, 1], f32)
        V.tensor_reduce(slog[:], lns[:], AX_X, ADD)
        lout = tiny.tile([128, 1], f32)
        V.tensor_scalar(lout[:], slog[:], -1.0, None, MULT)
        nc.sync.dma_start(loss_ap[:, :], lout[:])


def build_nc():
    nc = bass.Bass("TRN2", target_bir_lowering=False, debug=False)
    # t-chunk-major layout: row ch*BL + b holds y_pred[b, ch*CH:(ch+1)*CH, :]
    yp = nc.dram_tensor("yp16", [NCH * BL, CH, C], f16, kind="ExternalInput").ap()
    e_in = nc.dram_tensor("e_all", [128, BL * NS], f16, kind="ExternalInput").ap()
    m_in = nc.dram_tensor("m_odd", [128, 63], bf16, kind="ExternalInput").ap()
    loss = nc.dram_tensor("loss", [BL, 1], f32, kind="ExternalOutput").ap()
    with tile.TileContext(nc) as tc:
        _body(tc, loss, yp, e_in, m_in)
    return nc


def host_label_consts(y_true):
    """Per-core E [c, b*65] (64 label cols + blank, +eps) and skip mask."""
    import ml_dtypes

    lab = np.asarray(y_true).astype(np.int64)  # [B, L]
    outs = []
    for i in range(NCORES):
        lb = lab[i * BL : (i + 1) * BL]  # [128, 64]
        ext = np.concatenate(
            [lb, np.full((BL, 1), C - 1, np.int64)], axis=1
        )  # [128, 65]
        e = (np.arange(128)[:, None, None] == ext[None, :, :]).astype(np.float32)
        e = (e + EPS).astype(np.float16).reshape(128, BL * NS)
        # m[jj] = (lab[jj+1] != lab[jj]): skip allowed from source label jj
        m = (lb[:, 1:] != lb[:, :-1]).astype(ml_dtypes.bfloat16)  # [128, 63]
        outs.append((e, m))
    return outs


_CACHE = {}

# --- BIR legalizer -----------------------------------------------------------
# This container's walrus encodes at most ONE sync wait on SP-queue
# instruction classes (PSEUDO_DMA_DIRECT2D / XPOSE / CTRL): "Too many sync
# wait commands". Tile freely emits >=2 waits per instruction. Split the
# extras onto NoOps inserted just before (same engine stream => semantics
# preserved, waits satisfied in order).
_SPLIT_OPS = {"DMACopy", "DmaTransposeAnt", "DMAGatherAnt", "Drain", "NoOp"}


def _legalize_bir(bir_bytes):
    import orjson

    d = orjson.loads(bir_bytes)
    n_new = 0
    for fn in d.get("functions", []):
        for blk in fn.get("blocks", []):
            insts = blk.get("instructions")
            if not insts:
                continue
            out = []
            for ins in insts:
                si = ins.get("sync_info")
                if si:
                    waits = si.get("on_wait") or []
                    if len(waits) > 1:
                        for w in waits[:-1]:
                            n_new += 1
                            out.append(
                                {
                                    "debug": ins.get("debug", 0),
                                    "engine": ins["engine"],
                                    "ins": [],
                                    "outs": [],
                                    "name": f"ZW-{n_new}",
                                    "opcode": "NoOp",
                                    "sync_info": {"on_wait": [w], "on_update": []},
                                }
                            )
                        si["on_wait"] = [waits[-1]]
                out.append(ins)
            blk["instructions"] = out
    return orjson.dumps(d)


def _install_bir_legalizer():
    import concourse.bass2jax as b2j

    if getattr(b2j, "_ctc_legalizer_installed", False):
        return
    orig = b2j.compile_bir_kernel

    def wrapper(bir_json, tmpdir, neff_name="file.neff"):
        bir_json = _legalize_bir(bir_json)
        return orig(bir_json, tmpdir, neff_name=neff_name)

    b2j.compile_bir_kernel = wrapper
    b2j._ctc_legalizer_installed = True


def make_in_maps(y_true, y_pred):
    # f16 cast + t-chunk-major reorder: [B, T, C] -> [B, NCH, CH, C] -> per
    # core [NCH, BL, CH, C] flattened to [NCH*BL, CH, C]
    yp16 = np.asarray(y_pred, dtype=np.float16).reshape(B, NCH, CH, C)
    consts = host_label_consts(y_true)
    return [
        {
            "yp16": np.ascontiguousarray(
                yp16[i * BL : (i + 1) * BL].transpose(1, 0, 2, 3)
            ).reshape(NCH * BL, CH, C),
            "e_all": consts[i][0],
            "m_odd": consts[i][1],
        }
        for i in range(NCORES)
    ]


def kernel(y_true, y_pred):
    assert y_pred.shape == (B, T, C) and y_true.shape == (B, L)
    _install_bir_legalizer()
    nc = _CACHE.get("nc")
    if nc is None:
        nc = _CACHE["nc"] = build_nc()
    in_maps = make_in_maps(y_true, y_pred)
    res = run_bass_kernel_spmd(nc, in_maps, list(range(NCORES)))
    out = np.concatenate([res.results[i]["loss"] for i in range(NCORES)], axis=0)
    return out.astype(np.float32)
